# revision 31
# baseline (speedup 1.0000x reference)
"""DCN cross-layer kernel for Trainium2 (8 NeuronCores, data-parallel).

Reference computation (L=3 layers):
    x_{l+1} = x0 * (x_l . w_l) + b_l + x_l

Algebraic collapse used here: writing x_l = x0 * sigma_l + B_l, where
sigma_l is a per-row scalar and B_l = sum_{j<l} b_j is a broadcast
vector, the recurrence becomes
    d_l      = x0 . w_l                  (per-row dot, original x0!)
    sigma_{l+1} = sigma_l * (1 + d_l) + beta_l,   beta_l = B_l . w_l (host const)
    out      = x0 * sigma_3 + B_3
So the device kernel is one streaming pass over x.  Memory-bound.

bf16 I/O: x is uploaded as bf16 and out is stored as bf16 (host widens
back to f32).  Dot products accumulate in f32 PSUM and the sigma
recurrence runs in f32, so the only error sources are the input/output
roundings; measured end-to-end rel err ~4.5e-3 vs the f32 reference
(tolerance 2e-2).  HBM traffic per core: 2 MiB load + 2 MiB store
(vs 8 MiB for f32), halving the DMA-bus floor to ~11.7 us.

DMA structure (cost model: 360 GB/s bus shared by all transfers, plus
~625 ns/instruction of serialized HWDGE descriptor-gen): x loads are
mostly single-tile DMAs on the SP queue, issued before any store; the
first tile's load is split at the copy-group boundary (DVE-side chunks
first) so its dot chain starts two sem-props earlier; the tiny W load
rides the Pool/SWDGE path.  Stores are single-tile DMAs on SP, issued
as each tile's output multiply completes.

Engine split (per [128, 1024] tile, 728 ns/tile DMA-bus budget):
  PE   : 8 transposes (128x128, bf16 = 1 cycle/row) + 8 dot matmuls
         (K=128, M=128, N=3) + one K=1 ones-matmul that folds the +1
         into the PSUM accumulation, so PSUM holds e = 1 + d
  ACT  : PSUM->SBUF copy of transposed chunks 0..CCH-1
  DVE  : copy of the remaining chunks, sigma_3 = prod_l e_l as one
         free-dim multiply-reduce (PSUM->SBUF), and out = x0 * sigma_3
         cols [0, MUL_SPLIT) (tensor_scalar_mul, bf16 2x/4x mode)
  Pool : out-mul cols [MUL_SPLIT, D) on middle tiles, W-load descgen

TimelineSim (the grading cost model): 16330 ns/core vs 27304 ns for
the f32 version of the same pipeline (DMA-bus floor ~11.7 us + ~2.0 us
startup + ~1.5 us drain tails).
"""

import numpy as np

N_CORES = 8
B, D = 8192, 1024
L = 3
B_SH = B // N_CORES  # 1024 rows per core
P = 128
N_TILES = B_SH // P  # 8 tiles of [128, 1024] per core
N_CH = D // P        # 8 d-chunks per tile

# DMA batching (in units of 128-row tiles).  Small leading loads start
# compute early; small trailing stores shorten the final dependency
# chain.  Must each sum to N_TILES.
LOAD_GROUPS = [1, 1, 1, 1, 1, 1, 2]
STORE_GROUPS = [1, 1, 1, 1, 1, 1, 1, 1]

CCH = 4  # transpose chunks copied by ACT (rest by DVE)
# scheduler-clock stagger of tile prep (ns), see tile_wait_until use
PREP_OFF = 0
PREP_SLOPE = 0
ALT_STORE = False
DEP_SCHED = {}  # tile -> dep lag on sigma(t-lag)
ACT_BOTH = 0   # tiles whose DVE-side copy also goes to ACT (head warm-up)
MUL_SPLIT = 728  # Pool takes out-mul cols [MUL_SPLIT:D] on middle tiles (0=off)
POOL_X0 = False  # issue first x load via Pool/SWDGE (hurts: +3.6us)
X0_FAST = False  # issue first x load via ACT (shorter queue preamble)
SPLIT_X0 = 1  # first N load groups split at the copy-group boundary
SPLIT_LOAD = True  # actually split those loads (vs only swapping groups)
G1_FIRST = 0  # tiles with swapped (DVE-side-first) transpose groups
SPLIT_ST0 = 0  # col boundary to split tile 0's mul+store (0=off)
ACT_L1 = False  # issue tile 1's load from the ACT queue
FIRST_FINE = 0   # tiles using fine 2-chunk copy groups (head warm-up)

LAST_RESULTS = None  # BassKernelResults of the most recent run (for test.py)


def _build_program(betas, has_b3):
    import concourse.bacc as bacc
    import concourse.tile as tile
    from concourse import mybir
    from concourse.masks import make_identity

    f32 = mybir.dt.float32
    bf16 = mybir.dt.bfloat16
    mult = mybir.AluOpType.mult

    assert sum(LOAD_GROUPS) == N_TILES and sum(STORE_GROUPS) == N_TILES

    nc = bacc.Bacc("TRN2", target_bir_lowering=False, debug=False,
                   num_devices=N_CORES)

    x_d = nc.dram_tensor("x", [B_SH, D], bf16, kind="ExternalInput").ap()
    # host pre-arranges W^T as [p, c, l] so the load is 128 straight
    # 48-byte descriptors instead of a 1024-descriptor gather
    wt_d = nc.dram_tensor("wt", [P, N_CH * L], bf16, kind="ExternalInput").ap()
    out_d = nc.dram_tensor("out", [B_SH, D], bf16, kind="ExternalOutput").ap()
    b3_d = None
    if has_b3:
        b3_d = nc.dram_tensor("b3b", [P, D], bf16, kind="ExternalInput").ap()

    # [p, t, d] view: partition p of tile t holds x row t*128 + p
    x_v = x_d.rearrange("(t p) d -> p t d", p=P)
    out_v = out_d.rearrange("(t p) d -> p t d", p=P)

    with tile.TileContext(nc) as tc:
        with (
            tc.tile_pool(name="const", bufs=1) as const_pool,
            # full-depth SBUF pools: no slot reuse -> no false waits
            # chaining compute behind stores
            # group tiles have distinct tags (one use each) -> bufs=1
            tc.tile_pool(name="xin", bufs=1) as xin,
            tc.tile_pool(name="xtp", bufs=N_TILES) as xtp,
            tc.tile_pool(name="outp", bufs=1) as outp,
            tc.tile_pool(name="small", bufs=N_TILES) as small,
            tc.tile_pool(name="ptp", bufs=3, space="PSUM") as ptp,
            tc.tile_pool(name="pd", bufs=2, space="PSUM") as pd,
        ):
            ident = const_pool.tile([P, P], bf16, tag="ident")
            # ones row-vectors: a K=1 rank-1 matmul accumulates +1.0 into
            # every d[b, l], so PSUM holds e = 1 + d directly
            ones_m = const_pool.tile([1, P], bf16, tag="ones_m")
            nc.gpsimd.memset(ones_m[:], 1.0)
            ones_n = const_pool.tile([1, L], bf16, tag="ones_n")
            nc.gpsimd.memset(ones_n[:], 1.0)
            if has_b3:
                b3 = const_pool.tile([P, D], bf16, tag="b3")
                nc.gpsimd.dma_start(b3[:], b3_d[:])

            # x loads on the SP queue, batched per LOAD_GROUPS; issue
            # every load before any store so a compute op waiting for its
            # load never transitively waits on store completions
            xt_views = [None] * N_TILES  # per-tile [P, D] SBUF views
            wt_sb = None
            t0 = 0
            for gi, g in enumerate(LOAD_GROUPS):
                xg = xin.tile([P, g, D], bf16, tag=f"xg{gi}")
                if gi < SPLIT_X0 and SPLIT_LOAD:
                    # split tile 0's load at the copy-group boundary,
                    # DVE-side chunks first: each piece lands and clears
                    # its 900ns sem-prop earlier, pulling tile 0's
                    # transpose->dot->mul chain (and the first store) left
                    nc.sync.dma_start(xg[:, :, CCH * P:D],
                                      x_v[:, t0:t0 + g, CCH * P:D])
                    nc.sync.dma_start(xg[:, :, 0:CCH * P],
                                      x_v[:, t0:t0 + g, 0:CCH * P])
                else:
                    if gi == 1 and ACT_L1:
                        eng = nc.scalar
                    elif gi == 0 and X0_FAST:
                        eng = nc.gpsimd if POOL_X0 else nc.scalar
                    else:
                        eng = nc.sync
                    eng.dma_start(xg[:], x_v[:, t0:t0 + g, :])
                for j in range(g):
                    xt_views[t0 + j] = xg[:, j, :]
                t0 += g
                if gi == 0:
                    # tiny W^T load via Pool/SWDGE right after x tile 0
                    wt_sb = const_pool.tile([P, N_CH * L], bf16, tag="wt")
                    nc.gpsimd.dma_start(wt_sb[:], wt_d[:])
                    # identity (needed by the first transpose ~3.3us in)
                    # builds on Pool after the two urgent DMAs
                    make_identity(nc, ident[:])

            # store-group SBUF tiles, filled by the per-tile muls
            og_tiles = []
            store_of_tile = {}  # last tile index -> (group idx, tile0)
            t0 = 0
            for gi, g in enumerate(STORE_GROUPS):
                og_tiles.append(outp.tile([P, g, D], bf16,
                                          name=f"og{gi}", tag=f"og{gi}"))
                store_of_tile[t0 + g - 1] = (gi, t0)
                t0 += g

            sig_hist = []
            for t in range(N_TILES):
                xt = xt_views[t]

                # transpose the tile in two PSUM groups (chunks
                # [0,CCH) -> ACT copy, [CCH,8) -> DVE copy): one copy op
                # per engine, sized so ACT and DVE (which also runs the
                # reduce and its share of the mul) stay balanced under
                # the 728ns/tile bus budget.
                # xts[p, c*128+a] = xt[a, c*128+p]
                xts = xtp.tile([P, D], bf16, tag="xts")
                if t < FIRST_FINE:
                    # pipeline warm-up: 2-chunk groups, copies alternate
                    # ACT/DVE so tile 0's dot (hence first store) lands
                    # as early as possible
                    groups = ((0, 2), (2, 4), (4, 6), (6, 8))
                elif t < SPLIT_X0 or t < G1_FIRST:
                    # same groups, but emit the DVE-side one first to
                    # match its earlier-landing half-load
                    groups = ((CCH, N_CH), (0, CCH))
                else:
                    groups = ((0, CCH), (CCH, N_CH))
                # stagger tile t's prep in the scheduler's logical clock
                # so a later tile's transpose-copies never outrank an
                # earlier tile's dots->reduce->mul tail in the per-engine
                # ready heaps (the tail feeds the store stream)
                with tc.tile_wait_until(ms=(PREP_OFF + t * PREP_SLOPE) * 1e-6):
                    for (c0, c1) in groups:
                        w = (c1 - c0) * P
                        tpt = f"tp{0 if c0 in (0, 4) else CCH}"                             if t < FIRST_FINE else f"tp{c0}"
                        tp = ptp.tile([P, w], bf16, tag=tpt, name=tpt)
                        for j, c in enumerate(range(c0, c1)):
                            nc.tensor.transpose(
                                tp[:, j * P:(j + 1) * P],
                                xt[:, c * P:(c + 1) * P],
                                ident[:])
                        if (c0 // 2) % 2 == 0 if t < FIRST_FINE                                 else (c0 == 0 or t < ACT_BOTH):
                            nc.scalar.copy(xts[:, c0 * P:c1 * P], tp[:])
                        elif t not in DEP_SCHED:
                            nc.vector.tensor_copy(xts[:, c0 * P:c1 * P],
                                                  tp[:])
                        else:
                            # bypass ALU: out = in0, but the scalar operand
                            # adds a data edge on an earlier tile's sigma,
                            # keeping DVE's copies from running ahead of
                            # the reduce->mul tail during pipeline warm-up
                            nc.vector.tensor_scalar(
                                xts[:, c0 * P:c1 * P], tp[:],
                                sig_hist[t - DEP_SCHED[t]], None,
                                mybir.AluOpType.bypass)

                # d[b, l] = sum_d x0[b, d] W[l, d] via PE, accumulated
                # over the 8 d-chunks
                dps = pd.tile([P, L], f32, tag="dps")
                dcol = dps[:, 0:L]
                for c in range(N_CH):
                    nc.tensor.matmul(
                        dcol,
                        xts[:, c * P:(c + 1) * P],
                        wt_sb[:, c * L:(c + 1) * L],
                        start=(c == 0),
                        stop=False)
                nc.tensor.matmul(dcol, ones_m[:], ones_n[:],
                                 start=False, stop=True)

                # PSUM now holds e_l = 1 + d_l.  With all betas zero
                # (b == 0), sigma_3 = e_0*e_1*e_2: one multiply-reduce
                # along the free dim, PSUM -> SBUF.
                if not any(betas):
                    sigt = small.tile([P, 1], f32, tag="sigt")
                    nc.vector.tensor_reduce(
                        sigt[:], dcol, axis=mybir.AxisListType.X, op=mult)
                    sig = sigt[:]
                else:
                    # general recurrence sigma_{l+1} = sigma_l*e_l + beta_l
                    esb = small.tile([P, L], f32, tag="esb")
                    nc.vector.tensor_copy(esb[:], dcol)
                    if betas[0] != 0.0:
                        sig0 = small.tile([P, 1], f32, tag="sig1")
                        nc.vector.tensor_scalar_add(sig0[:], esb[:, 0:1],
                                                    float(betas[0]))
                        sig = sig0[:]
                    else:
                        sig = esb[:, 0:1]
                    for l in (1, 2):
                        nsig = small.tile([P, 1], f32, tag=f"sig{l + 1}")
                        nc.vector.tensor_tensor(
                            out=nsig[:], in0=sig, in1=esb[:, l:l + 1],
                            op=mult)
                        if betas[l] != 0.0:
                            nc.vector.tensor_scalar_add(
                                nsig[:], nsig[:], float(betas[l]))
                        sig = nsig[:]

                sig_hist.append(sig)

                # out = x0 * sigma_3 (+ B3) — bf16 fast-mode mul on DVE
                gi, gt0 = next((sgi, sgt0)
                               for tl, (sgi, sgt0) in store_of_tile.items()
                               if sgt0 <= t <= tl)
                ot = og_tiles[gi][:, t - gt0, :]
                if t == 0 and SPLIT_ST0:
                    # tile 0: split mul+store into column pieces so the
                    # first store bytes enter the bus as early as possible
                    h = SPLIT_ST0
                    nc.vector.tensor_scalar_mul(ot[:, 0:h], xt[:, 0:h], sig)
                    nc.vector.tensor_scalar_mul(ot[:, h:D], xt[:, h:D], sig)
                elif 0 < t < N_TILES - 1 and MUL_SPLIT:
                    # split the output mul with the idle Pool engine so
                    # DVE's per-tile total drops below the 728ns bus rate
                    # (first/last tile stay pure-DVE: Pool's ~1.1us mul
                    # would stretch the head/tail store latency)
                    nc.vector.tensor_scalar_mul(ot[:, 0:MUL_SPLIT],
                                                xt[:, 0:MUL_SPLIT], sig)
                    nc.gpsimd.tensor_scalar_mul(ot[:, MUL_SPLIT:D],
                                                xt[:, MUL_SPLIT:D], sig)
                else:
                    nc.vector.tensor_scalar_mul(ot, xt, sig)
                if has_b3:
                    nc.vector.tensor_add(ot, ot, b3[:])

                # store group complete -> issue its DMA on SP (loads
                # have drained HWDGE by the time the first store is
                # ready, so its 625ns descgen beats Pool/SWDGE's ~1040ns)
                if t in store_of_tile:
                    sgi, sgt0 = store_of_tile[t]
                    g = STORE_GROUPS[sgi]
                    eng = nc.sync if (sgi % 2 == 0 or not ALT_STORE) else nc.gpsimd
                    if t == 0 and SPLIT_ST0 and g == 1:
                        h = SPLIT_ST0
                        eng.dma_start(out_v[:, sgt0:sgt0 + g, 0:h],
                                      og_tiles[sgi][:, :, 0:h])
                        eng.dma_start(out_v[:, sgt0:sgt0 + g, h:D],
                                      og_tiles[sgi][:, :, h:D])
                    else:
                        eng.dma_start(out_v[:, sgt0:sgt0 + g, :],
                                      og_tiles[sgi][:])

    nc.compile()
    return nc


def predict_time_ns(trace_path=None):
    """Single-core timeline-sim of the kernel program (cost-model time in
    ns).  SPMD data-parallel with no collectives, so per-core time ==
    kernel time.  Optionally writes a perfetto trace."""
    from trails.perfetto import LazyPerfetto
    for _m in ("enable_explicit_ordering", "reserve_process_order",
               "add_counter", "add_flow", "add_instant"):
        if not hasattr(LazyPerfetto, _m):
            setattr(LazyPerfetto, _m, lambda self, *a, **k: None)
    from concourse.timeline_sim import TimelineSim

    nc = _build_program([0.0, 0.0, 0.0], False)
    tlsim = TimelineSim(nc, trace=trace_path is not None)
    tlsim.simulate()
    if trace_path is not None and tlsim.perfetto is not None:
        tlsim.perfetto.save(trace_path)
    return tlsim.time


def kernel(x, W, b):
    global LAST_RESULTS
    import ml_dtypes
    from concourse.bass_utils import run_bass_kernel_spmd

    bf16 = ml_dtypes.bfloat16
    x = np.asarray(x, dtype=np.float32)
    W = np.asarray(W, dtype=np.float32)
    b = np.asarray(b, dtype=np.float32)

    # Host precompute: beta_l = (sum_{j<l} b_j) . w_l  and B_3 = sum_l b_l.
    Bl = np.zeros(D, dtype=np.float64)
    betas = []
    for l in range(L):
        betas.append(float(Bl @ W[l].astype(np.float64)))
        Bl = Bl + b[l].astype(np.float64)
    B3 = Bl.astype(np.float32)
    has_b3 = bool(np.any(B3))

    nc = _build_program(betas, has_b3)

    x_bf = np.ascontiguousarray(x.astype(bf16))
    # wt[p, c*L + l] = W[l, c*128 + p]
    wt_host = np.ascontiguousarray(
        W.T.reshape(N_CH, P, L).transpose(1, 0, 2).reshape(P, N_CH * L)
        .astype(bf16))
    in_maps = []
    for i in range(N_CORES):
        m = {"x": x_bf[i * B_SH:(i + 1) * B_SH], "wt": wt_host}
        if has_b3:
            m["b3b"] = np.ascontiguousarray(
                np.broadcast_to(B3.astype(bf16), (P, D)))
        in_maps.append(m)

    res = run_bass_kernel_spmd(nc, in_maps, core_ids=list(range(N_CORES)))
    LAST_RESULTS = res
    out = np.concatenate([np.asarray(res.results[i]["out"])
                          for i in range(N_CORES)], axis=0)
    return out.astype(np.float32)

